# revision 16
# baseline (speedup 1.0000x reference)
"""Trainium2 Bass kernel for nn_CompProbModel_42691974922925.

Reference semantics: for each batch frame, the model builds a completion-
probability field over F=6600 field cells x NT=40 pass durations x P=10
players, then gathers a single row ``out = ind_pass[b_idx, tof, :]`` where
``b_idx`` (ball target cell) and ``tof`` (time-of-flight index) are scalars
derived from the frame. Exact dead-code elimination: the gathered row only
depends on the 40 trajectory cells ``path[b_idx, tof, s]`` (s = traj step),
so the live computation is a [40 steps x 10 players] problem:

    p[s,p]    = sigmoid(c * (T[tt_idx[s]] - t_tot(cell_s, player_p))) * lam_z[tof,s]
    q[s]      = max(1, sum_p p[s,p]);  pn = p / q
    all_t[s]  = sum_p pn[s,p]
    rem       = cumprod_s(1 - all_t);  shift = roll(rem, 1)  (row tof==0 => 1)
    out[p]    = sum_{s<=tof} shift[s] * pn[s,p] * lam_all[p]

Host side (numpy, f32-exact vs the jax reference): index math (tof, b_idx,
trajectory cell indices via round-half-even), gathering FIELD_LOCS rows and
packing operand blocks. Device side (Bass/Tile, per core): all the real
arithmetic - kinematics distances, sqrt/sigmoid, normalization, the exact
cumprod survival scan, and the final contraction as a PE matvec.

Device layout: partitions = players (10), free axis = trajectory step (40).
Player-dependent values ride as per-partition scalars, step-dependent values
as broadcast rows; the sum over players is a PE matmul against a ones
column, the cumprod over steps is a single free-axis scan instruction.

Sharding across the 8 NeuronCores: the live problem after the trajectory
reduction is tiny and sequential (cumprod over s), so inputs are replicated
and every core computes the full result redundantly; core 0's output is
returned. (The [F,40,40,P] field sweep the sharding hint describes is dead
code for the final gather, so there is nothing left worth splitting.)
"""

import numpy as np

f32 = np.float32
NX, NY, NT, P = 120, 55, 40, 10
F = NX * NY
G = 10.72468

# T_GRID = jnp.linspace(0.1, 4.0, 40, dtype=float32) - exact bits as produced
# by jax (identical on the CPU and neuron backends; np.linspace differs by
# 1 ulp at 6 entries, so the bit pattern is pinned here).
_TGRID_BITS = [
    0x3DCCCCCD, 0x3E4CCCCD, 0x3E99999A, 0x3ECCCCCD, 0x3F000000, 0x3F19999A,
    0x3F333334, 0x3F4CCCCD, 0x3F666667, 0x3F800000, 0x3F8CCCCD, 0x3F99999A,
    0x3FA66667, 0x3FB33334, 0x3FC00000, 0x3FCCCCCD, 0x3FD9999A, 0x3FE66666,
    0x3FF33333, 0x40000000, 0x40066667, 0x400CCCCD, 0x40133334, 0x4019999A,
    0x40200000, 0x40266667, 0x402CCCCD, 0x40333334, 0x4039999A, 0x40400000,
    0x40466667, 0x404CCCCD, 0x40533333, 0x4059999A, 0x40600000, 0x40666666,
    0x406CCCCD, 0x40733333, 0x4079999A, 0x40800000,
]
T_GRID = np.array(_TGRID_BITS, dtype=np.uint32).view(f32)

_x = np.linspace(0.5, 119.5, 120)
_y = np.linspace(-0.5, 53.5, 55)
_y[0] = -0.2
_yy, _xx = np.meshgrid(_y, _x, indexing="ij")
FIELD_LOCS = np.stack([_xx, _yy], -1).reshape(-1, 2).astype(f32)  # [6600,2]

# tt_idx[s] = round(10*T[s]-1): traj-step -> p_int time index (== arange(40)
# for these bits, but computed generically).
TT_IDX = np.round(f32(10.0) * T_GRID - f32(1.0)).astype(np.int32)

N_CORES = 8

# single packed device input [P, XCOLS] (f32); column map:
_C_CXY = 0      # 0:80    field-cell coords along traj (cx | cy), bcast over p
_C_RVXY = 80    # 80:160  player reaction velocities (rvx*40 | rvy*40)
_C_RLXY = 160   # 160:240 player reaction locations (rlx*40 | rly*40)
_C_SC = 240     # 240:256 sm, negsm, ninv_am, inv_am, two_am, neg_inv_sm,
                #         sigc, lam
_C_TG = 256     # 256:296 tgr[s] = T[tt_idx[s]] - reax (reax pre-folded)
_C_MLZ = 296    # 296:336 lam_z[tof, s] row
_C_MA = 336     # 336:376 tril-row mask * (tof!=0)
_C_MB = 376     # 376:416 tril-row mask * (tof==0)
_C_ONES = 416   # 416:427 ones (row/col slices for PE reductions)
XCOLS = 432

_CACHE = {}


def _build_program():
    """Build + compile the 8-core Bass program once per process."""
    import concourse.bacc as bacc
    import concourse.bass as bass
    import concourse.mybir as mybir
    import concourse.tile as tile

    dt = mybir.dt.float32
    op = mybir.AluOpType
    act = mybir.ActivationFunctionType

    nc = bacc.Bacc("TRN2", target_bir_lowering=False, debug=False,
                   num_devices=N_CORES)
    x_dram = nc.dram_tensor("xin", [P, XCOLS], dt, kind="ExternalInput")
    out_dram = nc.dram_tensor("res", [P, 1], dt, kind="ExternalOutput")

    with tile.TileContext(nc) as tc:
        with (
            tc.tile_pool(name="sb", bufs=1) as pool,
            tc.tile_pool(name="ps", bufs=1, space=bass.MemorySpace.PSUM) as psp,
        ):
            x = pool.tile([P, XCOLS], dt, name="x", tag="x")
            nc.sync.dma_start(x[:], x_dram.ap())

            cxy = x[:, _C_CXY:_C_CXY + 80]
            rlxy = x[:, _C_RLXY:_C_RLXY + 80]
            tgr = x[:, _C_TG:_C_TG + 40]
            mlz = x[:, _C_MLZ:_C_MLZ + 40]
            mAr = x[0:1, _C_MA:_C_MA + 40]
            mBr = x[0:1, _C_MB:_C_MB + 40]
            ones_r = x[0:1, _C_ONES:_C_ONES + 10]
            ones_c = x[:, _C_ONES + 10:_C_ONES + 11]
            sco = _C_SC
            sm, negsm = x[:, sco:sco + 1], x[:, sco + 1:sco + 2]
            ninv_am, inv_am = x[:, sco + 2:sco + 3], x[:, sco + 3:sco + 4]
            two_am, neg_inv_sm = x[:, sco + 4:sco + 5], x[:, sco + 5:sco + 6]
            sigc, lam = x[:, sco + 6:sco + 7], x[:, sco + 7:sco + 8]

            def wt(name, p=P, n=NT):
                return pool.tile([p, n], dt, name=name, tag=name)

            # dxy = cells - rloc, written in place over cxy. The squared
            # distances and velocity dots run as independent narrow pairs so
            # the DVE pipeline overlaps them (~102ns each vs 227ns wide).
            nc.vector.tensor_tensor(cxy, cxy, rlxy, op.subtract)
            dxy = x[:, _C_CXY:_C_CXY + 80]
            rvxy = x[:, _C_RVXY:_C_RVXY + 80]
            sq, nm = wt("sq", n=80), wt("nm", n=80)
            nc.vector.tensor_tensor(sq[:], dxy, dxy, op.mult)
            nc.vector.tensor_tensor(nm[:], dxy, rvxy, op.mult)
            d2t, num = wt("d2t"), wt("num")
            nc.vector.tensor_tensor(d2t[:], sq[:, 0:40], sq[:, 40:80], op.add)
            nc.vector.tensor_tensor(num[:], nm[:, 0:40], nm[:, 40:80], op.add)
            d2 = d2t[:]
            num = num[:]

            dmag, invd, s0 = wt("dmag"), wt("invd"), wt("s0")
            nc.scalar.sqrt(dmag[:], d2)
            nc.vector.reciprocal(invd[:], dmag[:])
            nc.vector.tensor_tensor(s0[:], num, invd[:], op.mult)
            nc.vector.tensor_scalar(s0[:], s0[:], sm, negsm, op.min, op.max)
            # top-speed-limited branch, am^2-scaled to shorten the chain:
            # alt = sqrt((s0/am)^2 + 2*dmag/am) - s0/am
            #     = sqrt(s0^2 + 2*dmag*am)/am - u
            u, s0sq, w1 = wt("u"), wt("s0sq"), wt("w1")
            nc.vector.tensor_scalar(u[:], s0[:], inv_am, None, op.mult)
            nc.vector.tensor_tensor(s0sq[:], s0[:], s0[:], op.mult)
            nc.vector.scalar_tensor_tensor(w1[:], dmag[:], two_am, s0sq[:],
                                           op.mult, op.add)
            rt, alt = wt("rt"), wt("alt")
            nc.scalar.sqrt(rt[:], w1[:])
            nc.vector.scalar_tensor_tensor(alt[:], rt[:], inv_am, u[:],
                                           op.mult, op.subtract)
            tlt, hb, dlt = wt("tlt"), wt("hb"), wt("dlt")
            # (s0-sm)*(-1/am) == (sm-s0)/am
            nc.vector.tensor_scalar(tlt[:], s0[:], sm, ninv_am,
                                    op.subtract, op.mult)
            nc.vector.tensor_scalar(hb[:], s0[:], sm, 0.5, op.add, op.mult)
            nc.vector.tensor_tensor(dlt[:], tlt[:], hb[:], op.mult)
            # exact select(dlt > dmag, alt, tlt): predicated overwrite with
            # a uint8 mask (walrus requires an integer mask dtype)
            gm = pool.tile([P, NT], mybir.dt.uint8, name="gm", tag="gm")
            nc.vector.tensor_tensor(gm[:], dlt[:], dmag[:], op.is_gt)
            nc.vector.copy_predicated(tlt[:], gm[:], alt[:])
            # dmag - clamp(dlt) == max(min(dmag-dlt, dmag), 0) exactly
            # (clamp lattice identity, valid since dmag >= 0); then
            # pm = (tgr - ee/sm) - tlt  with tgr = tg - reax host-folded
            ee, q1, pm = wt("ee"), wt("q1"), wt("pm")
            nc.vector.tensor_tensor(ee[:], dmag[:], dlt[:], op.subtract)
            nc.vector.scalar_tensor_tensor(ee[:], ee[:], 0.0, dmag[:],
                                           op.max, op.min)
            nc.vector.scalar_tensor_tensor(q1[:], ee[:], neg_inv_sm, tgr,
                                           op.mult, op.add)
            nc.vector.tensor_tensor(pm[:], q1[:], tlt[:], op.subtract)
            # p = sigmoid(sigc*pm); the lam_z window multiplies bit-exactly
            # into the row side instead (mlz is 0/1): s_sum = mlz*sum_p(pp),
            # and mlz rides in the host-prepared tril mask rows for the final
            # contraction, so pp feeds the PE reduction directly.
            pp = wt("pp")
            nc.scalar.activation(pp[:], pm[:], act.Sigmoid, scale=sigc)

            # s_sum over players (partition reduce on PE), then * lam_z row
            ps_sum = psp.tile([1, NT], dt, name="ps_sum", tag="ps_sum")
            nc.tensor.matmul(ps_sum[:], ones_c, pp[:])
            smul = pool.tile([1, NT], dt, name="smul", tag="smul")
            nc.vector.tensor_tensor(smul[:], ps_sum[:], mlz[0:1, :], op.mult)
            qrow = pool.tile([1, NT], dt, name="qrow", tag="qrow")
            invq = pool.tile([1, NT], dt, name="invq", tag="invq")
            nc.vector.tensor_scalar(qrow[:], smul[:], 1.0, None, op.max)
            nc.vector.reciprocal(invq[:], qrow[:])
            # survival factor (1 - all_t) == (qrow - s_sum)*invq, with the
            # invq multiply fused into the cumprod scan:
            # state = (A[t]*state)*invq[t]
            arow = pool.tile([1, NT], dt, name="arow", tag="arow")
            nc.vector.tensor_tensor(arow[:], qrow[:], smul[:], op.subtract)
            rem = pool.tile([1, NT], dt, name="rem", tag="rem")
            nc.vector.tensor_tensor_scan(rem[:], arow[:], invq[:], 1.0,
                                         op.mult, op.mult)
            # weff = (roll(rem,1)*mA + mB) * invq, shift folded into the
            # offset access patterns of the mA multiply
            # weff = roll(rem,1)*(mA*invq) + mB*invq; both mask*invq rows
            # depend only on invq so they compute off the critical path, and
            # the mB term joins via a second accumulating broadcast matmul.
            mbi = pool.tile([1, NT], dt, name="mbi", tag="mbi")
            nc.vector.tensor_tensor(mbi[:], mBr, invq[:], op.mult)
            mai = pool.tile([1, NT], dt, name="mai", tag="mai")
            nc.vector.tensor_tensor(mai[:], mAr, invq[:], op.mult)
            w2 = pool.tile([1, NT], dt, name="w2", tag="w2")
            nc.vector.tensor_tensor(w2[0:1, 1:NT], rem[0:1, 0:NT - 1],
                                    mai[0:1, 1:NT], op.mult)
            nc.vector.tensor_tensor(w2[0:1, 0:1], rem[0:1, NT - 1:NT],
                                    mai[0:1, 0:1], op.mult)
            # broadcast weff across partitions; fused final contraction:
            # res[p] = sum_s (ptraj[p,s]*lam[p]) * weff[s]
            ps_w = psp.tile([P, NT], dt, name="ps_w", tag="ps_w")
            nc.tensor.matmul(ps_w[:], ones_r, mbi[:], start=True, stop=False)
            nc.tensor.matmul(ps_w[:], ones_r, w2[:], start=False, stop=True)
            ind = wt("ind")
            res = pool.tile([P, 1], dt, name="res", tag="res")
            nc.vector.scalar_tensor_tensor(ind[:], pp[:], lam, ps_w[:],
                                           op.mult, op.mult,
                                           accum_out=res[:])
            nc.sync.dma_start(out_dram.ap(), res[:])

    nc.compile()
    return nc


def _get_nc():
    if "nc" not in _CACHE:
        _CACHE["nc"] = _build_program()
    return _CACHE["nc"]


def _host_prep(frame, s_max, a_max, tti_sigma, tti_lambda_off, tti_lambda_def):
    """Index math + operand packing for one batch element (numpy, f32)."""
    fr = np.asarray(frame, dtype=f32)[0]          # [P,13]
    sm = f32(np.asarray(s_max).reshape(-1)[0])
    am = f32(np.asarray(a_max).reshape(-1)[0])
    sig = f32(np.asarray(tti_sigma).reshape(-1)[0])
    lo = f32(np.asarray(tti_lambda_off).reshape(-1)[0])
    ld = f32(np.asarray(tti_lambda_def).reshape(-1)[0])

    reax = f32(sm / am)
    v_x_r = fr[:, 5] * reax + fr[:, 3]
    v_y_r = fr[:, 6] * reax + fr[:, 4]
    x_r = fr[:, 1] + fr[:, 3] * reax + f32(0.5) * fr[:, 5] * f32(reax * reax)
    y_r = fr[:, 2] + fr[:, 4] * reax + f32(0.5) * fr[:, 6] * f32(reax * reax)
    teams = fr[:, 7]
    rlx = x_r.astype(np.int32).astype(f32)        # trunc-toward-zero like jax
    rly = y_r.astype(np.int32).astype(f32)

    # scalar gathers (match jax negative-index wrap + OOB clip semantics)
    tof = int(np.round(fr[0, 12])) - 1
    if tof < 0:
        tof += NT
    tof = min(max(tof, 0), NT - 1)
    b_idx = (int(fr[0, 11]) + 1) * NX + int(fr[0, 10])
    if b_idx < 0:
        b_idx += F
    b_idx = min(max(b_idx, 0), F - 1)

    # ball trajectory for the (b_idx, tof) row; round-half-even like jnp.round
    ball = fr[0, 8:10]
    vx = f32((FIELD_LOCS[b_idx, 0] - ball[0]) / T_GRID[tof])
    vy = f32((FIELD_LOCS[b_idx, 1] - ball[1]) / T_GRID[tof])
    traj_x = np.round(
        np.minimum(np.maximum(ball[0] + vx * T_GRID, f32(0)), f32(NX - 1))
    ).astype(np.int32)
    traj_y = np.round(
        np.minimum(np.maximum(ball[1] + vy * T_GRID, f32(0)), f32(NY - 1))
    ).astype(np.int32)
    path = traj_y * NX + traj_x                    # [40] in-range by clip
    cells = FIELD_LOCS[path]                       # [40,2]

    # catchability window lam_z[tof, s]
    vz0_t = f32(T_GRID[tof] * f32(G) / f32(2.0))
    z_row = f32(2.0) + vz0_t * T_GRID - f32(0.5) * f32(G) * (T_GRID * T_GRID)
    mlz = ((z_row < f32(3.0)) & (z_row > f32(0.0))).astype(f32)

    msk = (np.arange(NT) <= tof).astype(f32)       # tril row tof
    is0 = 1.0 if tof == 0 else 0.0                 # t==0 row: shift forced to 1
    inv_am = f32(f32(1.0) / am)

    xin = np.zeros((P, XCOLS), f32)
    xin[:, _C_CXY:_C_CXY + 40] = cells[:, 0][None, :]
    xin[:, _C_CXY + 40:_C_CXY + 80] = cells[:, 1][None, :]
    xin[:, _C_RLXY:_C_RLXY + 40] = rlx[:, None]
    xin[:, _C_RLXY + 40:_C_RLXY + 80] = rly[:, None]
    xin[:, _C_RVXY:_C_RVXY + 40] = v_x_r[:, None]
    xin[:, _C_RVXY + 40:_C_RVXY + 80] = v_y_r[:, None]
    xin[:, _C_TG:_C_TG + 40] = (T_GRID[TT_IDX] - reax)[None, :]
    xin[:, _C_MLZ:_C_MLZ + 40] = mlz[None, :]
    xin[:, _C_MA:_C_MA + 40] = (msk * f32(1.0 - is0) * mlz)[None, :]
    xin[:, _C_MB:_C_MB + 40] = (msk * f32(is0) * mlz)[None, :]
    xin[:, _C_ONES:_C_ONES + 11] = 1.0
    sc = _C_SC
    xin[:, sc + 0], xin[:, sc + 1] = sm, -sm
    xin[:, sc + 2], xin[:, sc + 3] = -inv_am, inv_am
    xin[:, sc + 4], xin[:, sc + 5] = f32(2.0) * am, -(f32(1.0) / sm)
    xin[:, sc + 6] = f32(f32(3.14) / (f32(1.732) * sig))
    xin[:, sc + 7] = lo * teams + ld * (f32(1.0) - teams)
    return xin


def kernel(frame, s_max, a_max, tti_sigma, tti_lambda_off, tti_lambda_def):
    from concourse import bass_utils

    frame = np.asarray(frame, dtype=f32)
    B = frame.shape[0]
    nc = _get_nc()
    out = np.zeros((B, P), f32)
    for b in range(B):
        xin = _host_prep(frame[b:b + 1], s_max, a_max, tti_sigma,
                         tti_lambda_off, tti_lambda_def)
        in_maps = [{"xin": xin} for _ in range(N_CORES)]
        res = bass_utils.run_bass_kernel_spmd(nc, in_maps,
                                              core_ids=list(range(N_CORES)))
        out[b] = res.results[0]["res"][:, 0]
    return out
